# revision 1
# baseline (speedup 1.0000x reference)
"""FAVOR+ (Performer) causal linear attention on 8 Trainium2 NeuronCores.

Problem: B=2, L=2048, H=8, D=64, M=128 random features, fp32.
Sharding: the 16 (b,h) pairs are data-parallel; each of the 8 cores gets 2
pairs and runs the full feature-map + chunked causal scan for them with no
cross-core communication.

Math per (b,h) pair (C=128 position chunks, 16 chunks):
  q' = exp(c*q @ P^T - |c*q|^2/2 - rowmax) + EPS        (c = d^-1/4)
  k' = exp(c*k @ P^T - |c*k|^2/2 - globalmax) + EPS
  (the reference's ratio=1/sqrt(M) scaling cancels in num/den and is dropped)
  out_t = (sum_{s<=t} q'_t.k'_s * v_s) / (sum_{s<=t} q'_t.k'_s)
computed chunk-wise: intra-chunk via a masked [C,C] score matmul, cross-chunk
via a running KV ([M, D+1] with an appended ones column that carries the
denominator) accumulated in PSUM.

The stabilizers are applied OUTSIDE the exp: exp(dash) is computed unbiased
(max exponent ~22 for randn inputs, far below fp32 overflow), then
x' = exp(dash) * exp(-(diag+stab)) + EPS in one fused DVE tensor_scalar per
chunk, with the per-chunk exp(-(diag+stab)) columns produced in one batched
sub + exp per pair.

Host-side prep is layout-only: transposes / chunk-major rearranges so every
DMA moves >=4KB-contiguous runs, and a baked-in ones column on V that turns
the denominator into column 64 of the numerator matmuls.
"""

import numpy as np
from contextlib import ExitStack

import concourse.bass as bass
import concourse.mybir as mybir
from concourse import tile, masks
from concourse.bass_utils import run_bass_kernel_spmd

B, L, H, D, M = 2, 2048, 8, 64, 128
C = 128
NCH = L // C              # 16 chunks
E = D + 1                 # 65: value dim + denominator column
NCORES = 8
PPC = (B * H) // NCORES   # 2 (b,h) pairs per core
EPS = 1e-6
DN = 1.0 / (64.0 ** 0.25)       # data_normalizer
SQS = float(0.5 ** 0.5 * DN)    # Square(x*SQS) summed = |DN*x|^2/2
F32 = mybir.dt.float32
AX = mybir.AxisListType
OP = mybir.AluOpType
AF = mybir.ActivationFunctionType

_cache = {}


def _emit_k_phase(ctx, tc, pools, consts, p, qT, kT, qldp, kldp, vaugp, out):
    nc = tc.nc
    ident, mask_ut, ones_row, cPT = consts
    (big, small, io, scratch, pp128, pp65, kvps_pool, kvsb_pool) = pools

    cs = lambda c: slice(c * C, (c + 1) * C)
    cs64 = lambda c: slice(c * D, (c + 1) * D)
    cs65 = lambda c: slice(c * E, (c + 1) * E)

    # ---- per-pair bulk loads; K inputs first and split so the K-phase
    # matmuls can start after the first piece arrives ----
    HL = L // 2
    kT_sb = big.tile([D, L], F32, tag="kT")
    nc.sync.dma_start(kT_sb[:, 0:HL], kT[p][:, 0:HL])
    nc.sync.dma_start(kT_sb[:, HL:L], kT[p][:, HL:L])
    kld_sb = big.tile([C, NCH * D], F32, tag="kld")
    nc.sync.dma_start(kld_sb[:], kldp[p])
    vaug = big.tile([C, NCH * E], F32, tag="vaug")
    nc.sync.dma_start(vaug[:], vaugp[p])
    qT_sb = big.tile([D, L], F32, tag="qT")
    nc.sync.dma_start(qT_sb[:, 0:HL], qT[p][:, 0:HL])
    nc.sync.dma_start(qT_sb[:, HL:L], qT[p][:, HL:L])
    qld_sb = big.tile([C, NCH * D], F32, tag="qld")
    nc.sync.dma_start(qld_sb[:], qldp[p])
    out_all = big.tile([C, NCH * D], F32, tag="out_all")

    # ---- Phase K1: exp(k_dash) unbiased, running max, diag ----
    Kp_all = big.tile([128, L], F32, tag="kp")
    KpT_all = big.tile([128, L], F32, tag="kpt")
    kdiag = small.tile([128, NCH], F32, tag="kdiag")
    rmax = small.tile([128, 1], F32, tag="rmax")
    nc.any.memset(rmax[:], -3.0e38)
    for c in range(NCH):
        kdps = pp128.tile([C, M], F32, tag="pp128")
        nc.tensor.matmul(kdps[:], lhsT=kT_sb[:, cs(c)], rhs=cPT[:],
                         start=True, stop=True)
        nc.scalar.activation(Kp_all[:, cs(c)], kdps[:], AF.Exp)
        kmx = small.tile([128, 1], F32, tag="kmx")
        nc.vector.tensor_reduce(kmx[:], Kp_all[:, cs(c)], axis=AX.X, op=OP.max)
        nc.vector.tensor_max(rmax[:], rmax[:], kmx[:])
        scr = scratch.tile([C, D], F32, tag="scr")
        nc.gpsimd.tensor_mul(scr[:], kld_sb[:, cs64(c)], kld_sb[:, cs64(c)])
        nc.vector.tensor_reduce(kdiag[:, c:c + 1], scr[:], axis=AX.X, op=OP.add)

    # global stabilizer -> bcolk_all[:, c] = exp(-(diag_c + stab))
    rmx_ps = pp65.tile([1, 128], F32, tag="pp65")
    nc.tensor.transpose(rmx_ps[:], rmax[:], ident[:])
    gmax = small.tile([1, 1], F32, tag="gmax")
    nc.vector.tensor_reduce(gmax[:], rmx_ps[:], axis=AX.X, op=OP.max)
    rgm = small.tile([1, 1], F32, tag="rgm")
    nc.vector.reciprocal(rgm[:], gmax[:])
    nsb_ps = pp65.tile([128, 1], F32, tag="pp65")
    nc.tensor.matmul(nsb_ps[:], lhsT=ones_row[:], rhs=rgm[:], start=True, stop=True)
    rg = small.tile([128, 1], F32, tag="rg")
    nc.vector.tensor_copy(rg[:], nsb_ps[:])
    endk = small.tile([128, NCH], F32, tag="endk")
    nc.scalar.activation(endk[:], kdiag[:], AF.Exp, scale=-0.0625)
    bcolk = small.tile([128, NCH], F32, tag="bcolk")
    nc.vector.tensor_scalar_mul(bcolk[:], endk[:], rg[:])

    # ---- Phase K2: k' = exp(dash)*bcol + EPS, and its transpose ----
    for c in range(NCH):
        nc.gpsimd.tensor_scalar(out=Kp_all[:, cs(c)], in0=Kp_all[:, cs(c)],
                                scalar1=bcolk[:, c:c + 1], scalar2=EPS,
                                op0=OP.mult, op1=OP.add)
        ktps = pp128.tile([C, C], F32, tag="pp128")
        nc.tensor.transpose(ktps[:], Kp_all[:, cs(c)], ident[:])
        nc.scalar.copy(KpT_all[:, cs(c)], ktps[:])

    return (qT_sb, qld_sb, vaug, out_all, Kp_all, KpT_all)


def _emit_q_scan(ctx, tc, pools, consts, p, state, out):
    nc = tc.nc
    ident, mask_ut, ones_row, cPT = consts
    (big, small, io, scratch, pp128, pp65, kvps_pool, kvsb_pool) = pools
    (qT_sb, qld_sb, vaug, out_all, Kp_all, KpT_all) = state

    cs = lambda c: slice(c * C, (c + 1) * C)
    cs64 = lambda c: slice(c * D, (c + 1) * D)
    cs65 = lambda c: slice(c * E, (c + 1) * E)

    # ---- Phase Q: exp(q_dash) unbiased, rowmax, diag ----
    Qe_all = big.tile([128, L], F32, tag="qe")
    qdiag = small.tile([128, NCH], F32, tag="qdiag")
    emax = small.tile([128, NCH], F32, tag="emax")
    for c in range(NCH):
        qdps = pp128.tile([C, M], F32, tag="pp128")
        nc.tensor.matmul(qdps[:], lhsT=qT_sb[:, cs(c)], rhs=cPT[:],
                         start=True, stop=True)
        nc.scalar.activation(Qe_all[:, cs(c)], qdps[:], AF.Exp)
        # rowmax(exp) = exp(rowmax): reduce the SBUF exp copy instead of PSUM
        nc.vector.tensor_reduce(emax[:, c:c + 1], Qe_all[:, cs(c)], axis=AX.X,
                                op=OP.max)
        scrq = scratch.tile([C, D], F32, tag="scr")
        nc.gpsimd.tensor_mul(scrq[:], qld_sb[:, cs64(c)], qld_sb[:, cs64(c)])
        nc.vector.tensor_reduce(qdiag[:, c:c + 1], scrq[:], axis=AX.X, op=OP.add)
    # bcolq = exp(-diag) / emax
    end_ = small.tile([128, NCH], F32, tag="end")
    nc.scalar.activation(end_[:], qdiag[:], AF.Exp, scale=-0.0625)
    remax = small.tile([128, NCH], F32, tag="remax")
    nc.vector.reciprocal(remax[:], emax[:])
    bcolq = small.tile([128, NCH], F32, tag="bcolq")
    nc.vector.tensor_mul(bcolq[:], end_[:], remax[:])

    # q' = exp(dash)*bcol + EPS, transposed into QpT_all (chunk-independent)
    QpT_all = big.tile([128, L], F32, tag="qpt_all")
    for c in range(NCH):
        nc.gpsimd.tensor_scalar(out=Qe_all[:, cs(c)], in0=Qe_all[:, cs(c)],
                                scalar1=bcolq[:, c:c + 1], scalar2=EPS,
                                op0=OP.mult, op1=OP.add)
        qtps = pp128.tile([M, C], F32, tag="pp128")
        nc.tensor.transpose(qtps[:], Qe_all[:, cs(c)], ident[:])
        nc.scalar.copy(QpT_all[:, cs(c)], qtps[:])
    return state + (QpT_all,)


def _emit_scan(ctx, tc, pools, consts, p, state, out):
    nc = tc.nc
    ident, mask_ut, ones_row, cPT = consts
    (big, small, io, scratch, pp128, pp65, kvps_pool, kvsb_pool) = pools
    (qT_sb, qld_sb, vaug, out_all, Kp_all, KpT_all, QpT_all) = state

    cs = lambda c: slice(c * C, (c + 1) * C)
    cs64 = lambda c: slice(c * D, (c + 1) * D)
    cs65 = lambda c: slice(c * E, (c + 1) * E)

    # ---- causal scan: only the KV chain is serial now ----
    # dual PSUM accumulators (even/odd chunks) halve the serial depth;
    # the inter contribution is QpT @ (KV_even + KV_odd).
    kv_ps = [kvps_pool.tile([M, E], F32, tag="kvps", name=f"kvps_{p}_{i}")
             for i in range(2)]
    kv_sb = [None, None]
    for c in range(NCH):
        QpT = QpT_all[:, cs(c)]
        # S^T[j,i] = sum_m K'[j,m] Q'[i,m], then causal mask (keep j<=i)
        stps = pp128.tile([C, C], F32, tag="pp128")
        nc.tensor.matmul(stps[:], lhsT=KpT_all[:, cs(c)], rhs=QpT[:],
                         start=True, stop=True)
        stm = io.tile([C, C], F32, tag="stm")
        nc.vector.tensor_mul(stm[:], stps[:], mask_ut[:])

        # num_aug[i, 0:64]=attention numerator, [i,64]=denominator
        ops_ = pp65.tile([C, E], F32, tag="pp65")
        inters = [par for par in range(2) if kv_sb[par] is not None]
        nc.tensor.matmul(ops_[:], lhsT=stm[:], rhs=vaug[:, cs65(c)],
                         start=True, stop=(not inters))
        for n, par in enumerate(inters):
            nc.tensor.matmul(ops_[:], lhsT=QpT[:], rhs=kv_sb[par][:],
                             start=False, stop=(n == len(inters) - 1))

        # KV state += K'_c^T V_aug_c  (PSUM accumulation, even/odd banks)
        par = c % 2
        nc.tensor.matmul(kv_ps[par][:], lhsT=Kp_all[:, cs(c)],
                         rhs=vaug[:, cs65(c)],
                         start=(c < 2), stop=(c >= NCH - 2),
                         skip_group_check=True)
        if c < NCH - 1:
            kv_sb[par] = kvsb_pool.tile([M, E], F32, tag="kvsb",
                                         name=f"kvsb_{p}_{c}")
            nc.vector.tensor_copy(kv_sb[par][:], kv_ps[par][:])

        rc = small.tile([C, 1], F32, tag="rc")
        nc.vector.reciprocal(rc[:], ops_[:, D:E])
        nc.vector.tensor_scalar_mul(out_all[:, cs64(c)], ops_[:, 0:D], rc[:])

    nc.sync.dma_start(out[p], out_all[:])


def _kernel(ctx, tc, out, qT, kT, qldp, kldp, vaugp, projT):
    nc = tc.nc
    const = ctx.enter_context(tc.tile_pool(name="const", bufs=1))
    big = ctx.enter_context(tc.tile_pool(name="big", bufs=2))
    small = ctx.enter_context(tc.tile_pool(name="small", bufs=4))
    io = ctx.enter_context(tc.tile_pool(name="io", bufs=3))
    scratch = ctx.enter_context(tc.tile_pool(name="scratch", bufs=2))
    pp128 = ctx.enter_context(tc.tile_pool(name="pp128", bufs=3, space="PSUM"))
    pp65 = ctx.enter_context(tc.tile_pool(name="pp65", bufs=3, space="PSUM"))
    kvps_pool = ctx.enter_context(tc.tile_pool(name="kvps", bufs=2, space="PSUM"))
    kvsb_pool = ctx.enter_context(tc.tile_pool(name="kvsb", bufs=3))

    ident = const.tile([128, 128], F32)
    masks.make_identity(nc, ident[:])
    mask_ut = const.tile([128, 128], F32)
    masks.make_upper_triangular(nc, mask_ut[:], val=1.0, diag=True)
    ones_row = const.tile([1, 128], F32)
    nc.any.memset(ones_row[:], 1.0)
    projT_sb = const.tile([D, M], F32)
    nc.sync.dma_start(projT_sb[:], projT[:])
    cPT = const.tile([D, M], F32)
    nc.vector.tensor_scalar_mul(cPT[:], projT_sb[:], DN)

    pools = (big, small, io, scratch, pp128, pp65, kvps_pool, kvsb_pool)
    consts = (ident, mask_ut, ones_row, cPT)
    states = [
        _emit_k_phase(ctx, tc, pools, consts, p, qT, kT, qldp, kldp, vaugp, out)
        for p in range(PPC)
    ]
    for p in range(PPC):
        st = _emit_q_scan(ctx, tc, pools, consts, p, states[p], out)
        _emit_scan(ctx, tc, pools, consts, p, st, out)


def _split_multiwaits(nc):
    """The installed walrus encodes at most ONE semaphore wait per
    instruction (EventSemaphore excepted, which takes two).  Hoist extra
    wait conditions onto preceding EventSemaphores on the same engine —
    pure wait instructions, no pipeline flush."""
    fix_id = [0]

    def wait_ev(engine, waits):
        fix_id[0] += 1
        return mybir.InstEventSemaphore(
            name=f"I-waitfix-{fix_id[0]}",
            opcode="EventSemaphore",
            engine=engine,
            ins=[], outs=[],
            sync_info=mybir.SyncInfo(on_wait=list(waits), on_update=[]),
        )

    for fn in nc.m.functions:
        for blk in fn.blocks:
            new_insts = []
            for inst in blk.instructions:
                si = inst.sync_info
                waits = list(si.on_wait) if si is not None else []
                is_ev = type(inst).__name__ == "InstEventSemaphore"
                cap = 2 if is_ev else 1
                if len(waits) > cap:
                    extra, keep = waits[:-cap], waits[-cap:]
                    for i in range(0, len(extra), 2):
                        new_insts.append(wait_ev(inst.engine, extra[i:i + 2]))
                    si.on_wait = keep
                new_insts.append(inst)
            blk.instructions[:] = new_insts


def _build():
    if 'nc' in _cache:
        return _cache['nc']
    nc = bass.Bass("TRN2", target_bir_lowering=False, debug=False,
                   num_devices=NCORES)
    qT = nc.dram_tensor("qT", [PPC, D, L], F32, kind="ExternalInput").ap()
    kT = nc.dram_tensor("kT", [PPC, D, L], F32, kind="ExternalInput").ap()
    qldp = nc.dram_tensor("qldp", [PPC, C, NCH * D], F32, kind="ExternalInput").ap()
    kldp = nc.dram_tensor("kldp", [PPC, C, NCH * D], F32, kind="ExternalInput").ap()
    vaugp = nc.dram_tensor("vaugp", [PPC, C, NCH * E], F32, kind="ExternalInput").ap()
    projT = nc.dram_tensor("projT", [D, M], F32, kind="ExternalInput").ap()
    out = nc.dram_tensor("out", [PPC, C, NCH * D], F32, kind="ExternalOutput").ap()
    with tile.TileContext(nc) as tc:
        with ExitStack() as ctx:
            _kernel(ctx, tc, out, qT, kT, qldp, kldp, vaugp, projT)
    _split_multiwaits(nc)
    _cache['nc'] = nc
    return nc


def kernel(query, key, value, projection_matrix, _trace=False):
    """Full inputs in, full output out. Shards (b,h) pairs across 8 cores."""
    query = np.asarray(query, dtype=np.float32)
    key = np.asarray(key, dtype=np.float32)
    value = np.asarray(value, dtype=np.float32)
    projection_matrix = np.ascontiguousarray(
        np.asarray(projection_matrix, dtype=np.float32))

    nc = _build()

    # [B,L,H,D] -> [B*H, L, D] pair-major
    def pairs_ld(x):
        return np.ascontiguousarray(x.transpose(0, 2, 1, 3).reshape(B * H, L, D))

    # chunk-major [B*H, 128, NCH*D]: row p holds [chunk][d] for position p
    def chunkmaj(x_ld):
        return np.ascontiguousarray(
            x_ld.reshape(B * H, NCH, C, D).transpose(0, 2, 1, 3)
            .reshape(B * H, C, NCH * D))

    q_ld = pairs_ld(query)
    k_ld = pairs_ld(key)
    v_ld = pairs_ld(value)
    q_T = np.ascontiguousarray(q_ld.transpose(0, 2, 1))  # [B*H, D, L]
    k_T = np.ascontiguousarray(k_ld.transpose(0, 2, 1))
    q_cm = chunkmaj(q_ld)
    k_cm = chunkmaj(k_ld)
    # V with a baked ones column: [B*H, 128, NCH*(D+1)]
    v4 = v_ld.reshape(B * H, NCH, C, D).transpose(0, 2, 1, 3)  # [P,128,NCH,D]
    vaug = np.concatenate(
        [v4, np.ones((B * H, C, NCH, 1), dtype=np.float32)], axis=3)
    vaug = np.ascontiguousarray(vaug.reshape(B * H, C, NCH * E))

    in_maps = []
    for r in range(NCORES):
        sl = slice(r * PPC, (r + 1) * PPC)
        in_maps.append({
            "qT": q_T[sl], "kT": k_T[sl],
            "qldp": q_cm[sl], "kldp": k_cm[sl], "vaugp": vaug[sl],
            "projT": projection_matrix.T.copy(),
        })

    res = run_bass_kernel_spmd(nc, in_maps, list(range(NCORES)), trace=_trace)
    out_cm = np.empty((B * H, C, NCH * D), dtype=np.float32)
    for r in range(NCORES):
        out_cm[r * PPC:(r + 1) * PPC] = res.results[r]["out"]
    # chunk-major -> [B*H, L, D] -> [B, L, H, D]
    out_ld = out_cm.reshape(B * H, C, NCH, D).transpose(0, 2, 1, 3).reshape(
        B * H, L, D)
    full = out_ld.reshape(B, H, L, D).transpose(0, 2, 1, 3)
    if _trace:
        return np.ascontiguousarray(full), res
    return np.ascontiguousarray(full)

